# revision 1
# baseline (speedup 1.0000x reference)
"""Trainium2 Bass kernel for nn_CIN (xDeepFM compressed-interaction network).

Math: each CIN layer computes, per sample b and feature-dim d (a "column"
n=(b,d)):  y[o] = sum_{h,m} W[o,h,m] * a[h] * b[m]  — a bilinear form.

We avoid materializing the outer-product tensor z[h*m, n] (which needs slow
cross-partition broadcasts) by polarization:  a*b = ((a+b)^2 - a^2 - b^2)/2.
Each layer becomes:  s = V @ t   (pair sums, TensorE)
                     q = s*s     (elementwise square, ScalarE/VectorE)
                     y = C @ q + G @ t^2   (TensorE, PSUM-accumulated)
with V a 0/1 pair-selection matrix and C,G folded from W host-side (exact).

Layer 0 uses the symmetric fold (741 unordered pairs of 39 features);
layer 1 uses all 64*39=2496 (nh,x) pairs.  Everything on-device is fp16
(inputs/weights) with fp32 PSUM accumulation.

Sharding: pure data parallel — batch 4096 split as 512 per NeuronCore
across 8 cores; weights replicated.
"""

import numpy as np

B, F, D = 4096, 39, 16
L0, L1 = 128, 128
H1 = L0 // 2                      # 64 hidden maps feed layer 1
NCORES = 8
BL = B // NCORES                  # 512 samples per core
NCOL = BL * D                     # 8192 columns per core
NT = 512                          # columns per tile
NTILES = NCOL // NT               # 16
NB = NT // D                      # samples per tile (32)

K0 = F * (F - 1) // 2             # 741 layer-0 pairs
K1 = H1 * F                       # 2496 layer-1 pairs
T0 = F                            # t rows for layer 0 rhs (x)
T1 = 128                          # t rows: [x 0:39 | zeros 39:64 | nh 64:128]
NH0 = 64                          # nh base partition in t


def _chunks(k):
    out = []
    o = 0
    while o < k:
        c = min(128, k - o)
        out.append((o, c))
        o += k
        o = out[-1][0] + c
    return out


CH0 = _chunks(K0)                 # [(0,128)x5, (640,101)]
CH1 = _chunks(K1)                 # [(0,128)x19, (2432,64)]
NC0 = len(CH0)
NC1 = len(CH1)


def _host_weights(W0, b0, W1, b1):
    """Fold W0/W1 into the square-trick operands (all exact, fp32)."""
    W0 = np.asarray(W0, np.float32)
    W1 = np.asarray(W1, np.float32)
    S0 = W0.reshape(L0, F, F)
    S0 = (S0 + S0.transpose(0, 2, 1)) / 2
    iu = np.triu_indices(F, 1)                       # 741 (h<m) pairs
    V0 = np.zeros((K0, F), np.float32)
    V0[np.arange(K0), iu[0]] = 1
    V0[np.arange(K0), iu[1]] = 1
    C0 = S0[:, iu[0], iu[1]]                         # [128, 741]
    rowsum = S0.sum(2)
    G0 = np.einsum('ohh->oh', S0) * 2 - rowsum       # S[h,h] - sum_{m!=h} S[h,m]

    B1 = W1.reshape(L1, H1, F)
    hh, mm = np.meshgrid(np.arange(H1), np.arange(F), indexing='ij')
    hh, mm = hh.ravel(), mm.ravel()                  # 2496 pairs, h-major
    V1 = np.zeros((K1, T1), np.float32)
    V1[np.arange(K1), mm] = 1                        # x part at rows 0:39
    V1[np.arange(K1), NH0 + hh] = 1                  # nh part at rows 64:128
    C1 = B1[:, hh, mm] / 2                           # [128, 2496]
    G1 = np.zeros((L1, T1), np.float32)
    G1[:, :F] = -B1.sum(1) / 2                       # coeff on x^2
    G1[:, NH0:] = -B1.sum(2) / 2                     # coeff on nh^2

    def pack_stationary(Ct, chunks):
        # Ct: [K, 128] -> packed [128, 128*nchunks] fp16, chunk i in
        # partitions 0:kc, free cols i*128:(i+1)*128
        out = np.zeros((128, 128 * len(chunks)), np.float16)
        for i, (o, kc) in enumerate(chunks):
            out[:kc, i * 128:i * 128 + 128] = Ct[o:o + kc, :]
        return out

    def pad_cols(Vt, n):
        out = np.zeros((Vt.shape[0], n), np.float16)
        out[:, :Vt.shape[1]] = Vt
        return out

    return {
        "V0T": pad_cols(V0.T, 128 * NC0),            # [39, 768]
        "V1T": pad_cols(V1.T, 128 * NC1),            # [103, 2560]
        "C0T": pack_stationary(C0.T, CH0),           # [128, 768]
        "C1T": pack_stationary(C1.T, CH1),           # [128, 2560]
        "G0T": G0.T.astype(np.float16),              # [39, 128]
        "G1T": G1.T.astype(np.float16),              # [103, 128]
        "b0": np.asarray(b0, np.float32).reshape(L0, 1),
        "b1": np.asarray(b1, np.float32).reshape(L1, 1),
    }


_NC_CACHE = {}


def _build_nc(repeat=1):
    key = ("nc", repeat)
    if key in _NC_CACHE:
        return _NC_CACHE[key]
    from contextlib import ExitStack
    import concourse.bacc as bacc
    import concourse.mybir as mybir
    import concourse.tile as tile

    f16 = mybir.dt.float16
    f32 = mybir.dt.float32

    nc = bacc.Bacc("TRN2", target_bir_lowering=False, debug=False)

    xT_d = nc.dram_tensor("xT", [F, NCOL], f16, kind="ExternalInput")
    V0T_d = nc.dram_tensor("V0T", [F, 128 * NC0], f16, kind="ExternalInput")
    V1T_d = nc.dram_tensor("V1T", [T1, 128 * NC1], f16, kind="ExternalInput")
    C0T_d = nc.dram_tensor("C0T", [128, 128 * NC0], f16, kind="ExternalInput")
    C1T_d = nc.dram_tensor("C1T", [128, 128 * NC1], f16, kind="ExternalInput")
    G0T_d = nc.dram_tensor("G0T", [F, 128], f16, kind="ExternalInput")
    G1T_d = nc.dram_tensor("G1T", [T1, 128], f16, kind="ExternalInput")
    b0_d = nc.dram_tensor("b0", [L0, 1], f32, kind="ExternalInput")
    b1_d = nc.dram_tensor("b1", [L1, 1], f32, kind="ExternalInput")
    out_d = nc.dram_tensor("out", [L0 - H1 + L1, BL], f32, kind="ExternalOutput")

    Relu = mybir.ActivationFunctionType.Relu

    with tile.TileContext(nc) as tc, ExitStack() as ctx:
        const = ctx.enter_context(tc.tile_pool(name="const", bufs=1))
        tp = ctx.enter_context(tc.tile_pool(name="tp", bufs=2))
        t2p = ctx.enter_context(tc.tile_pool(name="t2p", bufs=2))
        sq0p = ctx.enter_context(tc.tile_pool(name="sq0p", bufs=2))
        sq1p = ctx.enter_context(tc.tile_pool(name="sq1p", bufs=2))
        rp = ctx.enter_context(tc.tile_pool(name="rp", bufs=2))
        outp = ctx.enter_context(tc.tile_pool(name="outp", bufs=1))
        sps = ctx.enter_context(tc.tile_pool(name="sps", bufs=4, space="PSUM"))
        yps0 = ctx.enter_context(tc.tile_pool(name="yps0", bufs=2, space="PSUM"))
        yps1 = ctx.enter_context(tc.tile_pool(name="yps1", bufs=2, space="PSUM"))

        # resident weights
        V0T = const.tile([F, 128 * NC0], f16)
        V1T = const.tile([T1, 128 * NC1], f16)
        C0T = const.tile([128, 128 * NC0], f16)
        C1T = const.tile([128, 128 * NC1], f16)
        G0T = const.tile([F, 128], f16)
        G1T = const.tile([T1, 128], f16)
        b0t = const.tile([L0, 1], f32)
        b1t = const.tile([L1, 1], f32)
        for dst, src in ((V0T, V0T_d), (V1T, V1T_d), (C0T, C0T_d),
                         (C1T, C1T_d), (G0T, G0T_d), (G1T, G1T_d),
                         (b0t, b0_d), (b1t, b1_d)):
            nc.sync.dma_start(out=dst[:], in_=src.ap())

        out0 = outp.tile([H1, BL], f32)
        out1 = outp.tile([L1, BL], f32)

        for nt in [nt for _ in range(repeat) for nt in range(NTILES)]:
            csl = slice(nt * NT, (nt + 1) * NT)
            # t = [x (0:39); nh (39:103)]
            t = tp.tile([T1, NT], f16)
            t2 = t2p.tile([T1, NT], f16)
            nc.vector.memset(t[32:NH0, :], 0.0)              # zero pad rows
            nc.vector.memset(t2[32:NH0, :], 0.0)
            nc.sync.dma_start(out=t[0:F, :], in_=xT_d.ap()[:, csl])
            nc.scalar.square(t2[0:F, :], t[0:F, :])          # x^2

            # ---- layer 0: s0 = V0 @ x ; sq0 = s0^2 ----
            sq0 = sq0p.tile([128, NC0 * NT], f16)
            for i, (o, kc) in enumerate(CH0):
                ps = sps.tile([128, NT], f32)
                nc.tensor.matmul(ps[0:kc, :], V0T[:, i * 128:i * 128 + kc],
                                 t[0:F, :], start=True, stop=True)
                dst = sq0[0:kc, i * NT:(i + 1) * NT]
                if i % 5 in (1, 3):
                    nc.vector.tensor_copy(dst, ps[0:kc, :])
                    nc.vector.tensor_mul(dst, dst, dst)
                else:
                    nc.scalar.square(dst, ps[0:kc, :])

            # ---- y0 = C0 @ sq0 + G0 @ x^2 ----
            y0 = yps0.tile([L0, NT], f32)
            for i, (o, kc) in enumerate(CH0):
                nc.tensor.matmul(y0[:], C0T[0:kc, i * 128:(i + 1) * 128],
                                 sq0[0:kc, i * NT:(i + 1) * NT],
                                 start=(i == 0), stop=False)
            nc.tensor.matmul(y0[:], G0T[:], t2[0:F, :], start=False, stop=True)

            # relu + split
            nc.scalar.activation(t[NH0:T1, :], y0[0:H1, :], Relu, bias=b0t[0:H1])
            r0 = rp.tile([H1, NT], f32, tag="r0")
            nc.scalar.activation(r0[:], y0[H1:L0, :], Relu, bias=b0t[H1:L0])
            nc.scalar.square(t2[NH0:T1, :], t[NH0:T1, :])    # nh^2

            # ---- layer 1: s1 = V1 @ [x; nh] ; sq1 = s1^2 ----
            sq1 = sq1p.tile([128, NC1 * NT], f16)
            for i, (o, kc) in enumerate(CH1):
                ps = sps.tile([128, NT], f32)
                nc.tensor.matmul(ps[0:kc, :], V1T[:, i * 128:i * 128 + kc],
                                 t[:], start=True, stop=True)
                dst = sq1[0:kc, i * NT:(i + 1) * NT]
                if i % 5 in (1, 3):
                    nc.vector.tensor_copy(dst, ps[0:kc, :])
                    nc.vector.tensor_mul(dst, dst, dst)
                else:
                    nc.scalar.square(dst, ps[0:kc, :])

            # ---- y1 = C1 @ sq1 + G1 @ t^2 ----
            y1 = yps1.tile([L1, NT], f32)
            for i, (o, kc) in enumerate(CH1):
                nc.tensor.matmul(y1[:], C1T[0:kc, i * 128:(i + 1) * 128],
                                 sq1[0:kc, i * NT:(i + 1) * NT],
                                 start=(i == 0), stop=False)
            nc.tensor.matmul(y1[:], G1T[:], t2[:], start=False, stop=True)

            r1 = rp.tile([L1, NT], f32, tag="r1")
            nc.scalar.activation(r1[:], y1[:], Relu, bias=b1t[:])

            # ---- sum over d (innermost 16 of each column group) ----
            bsl = slice(nt * NB, (nt + 1) * NB)
            nc.vector.tensor_reduce(
                out0[:, bsl], r0[:].rearrange("p (b d) -> p b d", d=D),
                axis=mybir.AxisListType.X, op=mybir.AluOpType.add)
            nc.vector.tensor_reduce(
                out1[:, bsl], r1[:].rearrange("p (b d) -> p b d", d=D),
                axis=mybir.AxisListType.X, op=mybir.AluOpType.add)

        nc.sync.dma_start(out=out_d.ap()[0:H1, :], in_=out0[:])
        nc.sync.dma_start(out=out_d.ap()[H1:, :], in_=out1[:])

    nc.compile()
    _NC_CACHE[key] = nc
    return nc


def _run(inputs, trace=False):
    from concourse.bass_utils import run_bass_kernel_spmd

    x = np.asarray(inputs["x"], np.float32)
    w = _host_weights(inputs["W0"], inputs["b0"], inputs["W1"], inputs["b1"])
    nc = _build_nc()

    in_maps = []
    for c in range(NCORES):
        xs = x[c * BL:(c + 1) * BL]                          # [512, 39, 16]
        xT = np.ascontiguousarray(
            xs.transpose(1, 0, 2).reshape(F, NCOL)).astype(np.float16)
        m = {"xT": xT}
        m.update(w)
        in_maps.append(m)

    res = run_bass_kernel_spmd(nc, in_maps, core_ids=list(range(NCORES)),
                               trace=trace)
    out = np.empty((B, L0 - H1 + L1), np.float32)
    for c in range(NCORES):
        out[c * BL:(c + 1) * BL] = res.results[c]["out"].T
    return out, res


def kernel(**inputs):
    out, _ = _run(inputs)
    return out



# revision 38
# speedup vs baseline: 1.2690x; 1.2690x over previous
"""Trainium2 Bass kernel for nn_CIN (xDeepFM compressed-interaction network).

Math: each CIN layer computes, per sample b and feature-dim d (a "column"
n=(b,d)):  y[o] = sum_{h,m} W[o,h,m] * a[h] * b[m]  — a bilinear form.

We avoid materializing the outer-product tensor z[h*m, n] (which needs slow
cross-partition broadcasts) by polarization:  a*b = ((a+b)^2 - a^2 - b^2)/2.
Each layer becomes:  s = V @ t   (pair sums, TensorE)
                     q = s*s     (elementwise square, ScalarE/VectorE)
                     y = C @ q + G @ t^2   (TensorE, PSUM-accumulated)
with V a 0/1 pair-selection matrix and C,G folded from W host-side (exact).

Layer 0 uses the symmetric fold (741 unordered pairs of 39 features);
layer 1 uses all 64*39=2496 (nh,x) pairs.  Everything on-device is fp16
(inputs/weights) with fp32 PSUM accumulation.

Schedule: software-pipelined across column tiles so TensorE never waits.
Per iteration k the PE stream is  V0(k) | V1(k-1) ⋈ [C0(k),G0(k),C1(k-1),
G1(k-1)]  (1:1 interleave of producer and consumer passes).  Squares are
split between ScalarE (odd chunks, activation-square straight from PSUM)
and VectorE (even chunks, tensor_mul(ps,ps)); relu/bias on ScalarE; the
d-axis reduction on VectorE.  PSUM budget: 6-deep ps ring + y0 + y1 =
8 banks exactly.

Sharding: pure data parallel — batch 4096 split as 512 per NeuronCore
across 8 cores; weights replicated.
"""

import numpy as np

B, F, D = 4096, 39, 16
L0, L1 = 128, 128
H1 = L0 // 2                      # 64 hidden maps feed layer 1
NCORES = 8
BL = B // NCORES                  # 512 samples per core
NCOL = BL * D                     # 8192 columns per core
NT = 512                          # columns per tile
NTILES = NCOL // NT               # 16
NB = NT // D                      # samples per tile (32)

K0 = F * (F - 1) // 2             # 741 layer-0 pairs
K1 = H1 * F                       # 2496 layer-1 pairs
T0 = F                            # t rows for layer 0 rhs (x)
T1 = 128                          # t rows: [x 0:39 | zeros 39:64 | nh 64:128]
NH0 = 64                          # nh base partition in t


def _chunks(k):
    out = []
    o = 0
    while o < k:
        c = min(128, k - o)
        out.append((o, c))
        o += k
        o = out[-1][0] + c
    return out


CH0 = _chunks(K0)                 # [(0,128)x5, (640,101)]
CH1 = _chunks(K1 + NH0)           # [(0,128)x20] — last 64 rows are nh^2
NC0 = len(CH0)
NC1 = len(CH1)


def _host_weights(W0, b0, W1, b1):
    """Fold W0/W1 into the square-trick operands (all exact, fp32)."""
    W0 = np.asarray(W0, np.float32)
    W1 = np.asarray(W1, np.float32)
    S0 = W0.reshape(L0, F, F)
    S0 = (S0 + S0.transpose(0, 2, 1)) / 2
    iu = np.triu_indices(F, 1)                       # 741 (h<m) pairs
    V0 = np.zeros((K0, F), np.float32)
    V0[np.arange(K0), iu[0]] = 1
    V0[np.arange(K0), iu[1]] = 1
    C0 = S0[:, iu[0], iu[1]]                         # [128, 741]
    rowsum = S0.sum(2)
    G0 = np.einsum('ohh->oh', S0) * 2 - rowsum       # S[h,h] - sum_{m!=h} S[h,m]

    B1 = W1.reshape(L1, H1, F)
    hh, mm = np.meshgrid(np.arange(H1), np.arange(F), indexing='ij')
    hh, mm = hh.ravel(), mm.ravel()                  # 2496 pairs, h-major
    # chunk 19 carries 64 extra pass-through rows (identity on nh) whose
    # squares are nh^2, so G1's nh^2 term rides in C1 and the separate
    # nh^2 elementwise op disappears.
    V1 = np.zeros((K1 + NH0, T1), np.float32)
    V1[np.arange(K1), mm] = 1                        # x part at rows 0:39
    V1[np.arange(K1), NH0 + hh] = 1                  # nh part at rows 64:128
    V1[K1 + np.arange(NH0), NH0 + np.arange(NH0)] = 1
    C1 = np.concatenate([B1[:, hh, mm] / 2,          # [128, 2496]
                         -B1.sum(2) / 2], axis=1)    # nh^2 coeffs [128, 64]
    G1 = (-B1.sum(1) / 2)                            # x^2 coeffs [128, 39]

    def pack_stationary(Ct, chunks):
        # Ct: [K, 128] -> packed [128, 128*nchunks] fp16, chunk i in
        # partitions 0:kc, free cols i*128:(i+1)*128
        out = np.zeros((128, 128 * len(chunks)), np.float16)
        for i, (o, kc) in enumerate(chunks):
            out[:kc, i * 128:i * 128 + 128] = Ct[o:o + kc, :]
        return out

    def pad_cols(Vt, n):
        out = np.zeros((Vt.shape[0], n), np.float16)
        out[:, :Vt.shape[1]] = Vt
        return out

    return {
        "V0T": pad_cols(V0.T, 128 * NC0),            # [39, 768]
        "V1T": pad_cols(V1.T, 128 * NC1),            # [128, 2560]
        "C0T": pack_stationary(C0.T, CH0),           # [128, 768]
        "C1T": pack_stationary(C1.T, CH1),           # [128, 2560]
        "G0T": G0.T.astype(np.float16),              # [39, 128]
        "G1T": G1.T.astype(np.float16),              # [39, 128]
        "b0": np.asarray(b0, np.float32).reshape(L0, 1),
        "b1": np.asarray(b1, np.float32).reshape(L1, 1),
    }


_NC_CACHE = {}


def _build_nc():
    key = "nc"
    if key in _NC_CACHE:
        return _NC_CACHE[key]
    from contextlib import ExitStack
    import concourse.bacc as bacc
    import concourse.mybir as mybir
    import concourse.tile as tile

    f16 = mybir.dt.float16
    f32 = mybir.dt.float32

    nc = bacc.Bacc("TRN2", target_bir_lowering=False, debug=False)

    xT_d = nc.dram_tensor("xT", [F, NCOL], f16, kind="ExternalInput")
    V0T_d = nc.dram_tensor("V0T", [F, 128 * NC0], f16, kind="ExternalInput")
    V1T_d = nc.dram_tensor("V1T", [T1, 128 * NC1], f16, kind="ExternalInput")
    C0T_d = nc.dram_tensor("C0T", [128, 128 * NC0], f16, kind="ExternalInput")
    C1T_d = nc.dram_tensor("C1T", [128, 128 * NC1], f16, kind="ExternalInput")
    G0T_d = nc.dram_tensor("G0T", [F, 128], f16, kind="ExternalInput")
    G1T_d = nc.dram_tensor("G1T", [F, 128], f16, kind="ExternalInput")
    b0_d = nc.dram_tensor("b0", [L0, 1], f32, kind="ExternalInput")
    b1_d = nc.dram_tensor("b1", [L1, 1], f32, kind="ExternalInput")
    out_d = nc.dram_tensor("out", [L0 - H1 + L1, BL], f16, kind="ExternalOutput")

    Relu = mybir.ActivationFunctionType.Relu

    # Engine assignment for the 26 per-tile squares (global index:
    # sq0 chunk i -> i, sq1 chunk i -> 6+i).  'A': ScalarE activation-square
    # straight from PSUM.  'D': VectorE copy to SBUF + in-place mul (PSUM
    # may only feed one DVE operand).  'P': VectorE copy + GPSIMD in-place
    # mul (only for chunks with long producer->consumer slack).
    SQ_ENG = {}
    for g in range(6):
        SQ_ENG[g] = 'D' if g % 2 == 0 else 'A'
    for i in range(20):
        SQ_ENG[6 + i] = 'D' if i in (0, 2, 5, 7, 10, 12, 15, 17) else 'A'

    with tile.TileContext(nc) as tc, ExitStack() as ctx:
        const = ctx.enter_context(tc.tile_pool(name="const", bufs=1))
        sqp = ctx.enter_context(tc.tile_pool(name="sqp", bufs=2))
        rp = ctx.enter_context(tc.tile_pool(name="rp", bufs=2))
        redp = ctx.enter_context(tc.tile_pool(name="redp", bufs=2))
        sps = ctx.enter_context(tc.tile_pool(name="sps", bufs=6, space="PSUM"))
        yps = ctx.enter_context(tc.tile_pool(name="yps", bufs=1, space="PSUM"))

        def emit_square(dst, ps, kc, g):
            eng = SQ_ENG[g]
            if eng == 'A':
                nc.scalar.square(dst, ps[0:kc, :])
            else:
                nc.vector.tensor_copy(dst, ps[0:kc, :])
                mul = nc.vector.tensor_mul if eng == 'D' else \
                    nc.gpsimd.tensor_mul
                mul(dst, dst, dst)

        def emit_reduce(out_ap, r, rows, tag, fast=False):
            # d-axis sum of relu'd maps; log2 add-tree on idle GPSIMD, or
            # a single DVE reduce when the result is on the drain path
            if fast:
                with nc.allow_low_precision(reason="16-term d-sum fits fp16"):
                    nc.vector.tensor_reduce(
                        out_ap, r[:].rearrange("p (b d) -> p b d", d=D),
                        axis=mybir.AxisListType.X, op=mybir.AluOpType.add)
                return
            v = r[:].rearrange("p (b d) -> p b d", d=D)
            for w in (8, 4, 2):
                tmp = redp.tile([rows, NB * w], f16, tag=f"{tag}{w}",
                                name=f"{tag}{w}")
                tv = tmp[:].rearrange("p (b d) -> p b d", d=w)
                nc.gpsimd.tensor_add(tv, v[:, :, 0:w], v[:, :, w:2 * w])
                v = tv
            nc.gpsimd.tensor_add(out_ap.rearrange("p (b d) -> p b d", d=1),
                                 v[:, :, 0:1], v[:, :, 1:2])

        # Weight loads, ordered so the first V0/C0 passes aren't gated on
        # the big layer-1 operands.
        V0T = const.tile([F, 128 * NC0], f16)
        nc.sync.dma_start(out=V0T[:], in_=V0T_d.ap())

        # x / x^2 working buffers (3-deep manual rotation; zero pad rows
        # 39:64 are written exactly once, here, off the critical engines).
        tb = [const.tile([T1, NT], f16, name=f"tb{i}") for i in range(3)]
        t2b = [const.tile([F, NT], f16, name=f"t2b{i}") for i in range(3)]
        for i in (2, 0, 1):
            # partition offsets must be 32-aligned; rows 32:39 are
            # overwritten by every x DMA, rows 39:64 stay zero forever.
            # tb[2] first: the warmup matmuls below read it, and its x DMA
            # comes latest.
            nc.gpsimd.memset(tb[i][32:NH0, :], 0.0)
        nc.sync.dma_start(out=tb[0][0:F, :], in_=xT_d.ap()[:, 0:NT])
        nc.sync.dma_start(out=tb[1][0:F, :], in_=xT_d.ap()[:, NT:2 * NT])

        C0T = const.tile([128, 128 * NC0], f16)
        nc.sync.dma_start(out=C0T[:], in_=C0T_d.ap())
        G0T = const.tile([F, 128], f16)
        nc.sync.dma_start(out=G0T[:], in_=G0T_d.ap())
        b0t = const.tile([L0, 1], f32)
        nc.sync.dma_start(out=b0t[:], in_=b0_d.ap())

        # Layer-1 weights are ~1.3 MB; split the transfers and order them
        # by first use so early passes aren't gated on the whole block.
        HC = 128 * NC1 // 2
        V1T = const.tile([T1, 128 * NC1], f16)
        C1T = const.tile([128, 128 * NC1], f16)
        nc.sync.dma_start(out=V1T[:, 0:HC], in_=V1T_d.ap()[:, 0:HC])
        nc.sync.dma_start(out=C1T[:, 0:HC], in_=C1T_d.ap()[:, 0:HC])
        nc.sync.dma_start(out=V1T[:, HC:], in_=V1T_d.ap()[:, HC:])
        nc.sync.dma_start(out=C1T[:, HC:], in_=C1T_d.ap()[:, HC:])
        G1T = const.tile([F, 128], f16)
        nc.sync.dma_start(out=G1T[:], in_=G1T_d.ap())
        b1t = const.tile([L1, 1], f32)
        nc.sync.dma_start(out=b1t[:], in_=b1_d.ap())

        nc.vector.tensor_mul(t2b[0][:], tb[0][0:F, :], tb[0][0:F, :])

        out0 = const.tile([H1, BL], f16)
        out1 = const.tile([L1, BL], f16)

        # Warmup matmuls on the zeroed pad rows: keeps PE continuously busy
        # through the input-DMA latency so the p-state ramp (0.65 GHz ->
        # 2.4 GHz after 3us of uninterrupted work) is already paid before
        # real work arrives.  Results land in the y1 PSUM slot, which is
        # overwritten (start=True) before its first real use.
        warm = yps.tile([32, NT], f32, tag="y1", name="warm")
        for _ in range(6):
            nc.tensor.matmul(warm[:], tb[2][32:NH0, 0:32],
                             tb[2][32:NH0, :], start=True, stop=True)

        r0_ref = {}
        r1_ref = {}
        sq0_ref = {}
        sq1_ref = {}
        y0_ref = {}
        y1_ref = {}

        for k in range(NTILES + 2):
            cur = k if k < NTILES else None
            prev = k - 1 if 1 <= k <= NTILES else None

            # ---- S1: V0(cur) + inline squares ----
            if cur is not None:
                t = tb[cur % 3]
                sq0 = sqp.tile([128, NC0 * NT], f16, tag="sq0", name="sq0")
                sq0_ref[cur] = sq0
                for i, (o, kc) in enumerate(CH0):
                    ps = sps.tile([128, NT], f32, tag="ps", name="ps")
                    nc.tensor.matmul(ps[0:kc, :], V0T[:, i * 128:i * 128 + kc],
                                     t[0:F, :], start=True, stop=True)
                    emit_square(sq0[0:kc, i * NT:(i + 1) * NT], ps, kc, i)
                y0_ref[cur] = yps.tile([L0, NT], f32, tag="y0", name="y0")

            # ---- d-axis reductions of finished tiles ----
            # stream finished output columns out in blocks, keeping the
            # final (drain-path) block tiny
            blocks0 = {3: slice(0, 4 * NB), 7: slice(4 * NB, 8 * NB),
                       11: slice(8 * NB, 12 * NB),
                       15: slice(12 * NB, 16 * NB)}
            blocks1 = {3: slice(0, 4 * NB), 7: slice(4 * NB, 8 * NB),
                       11: slice(8 * NB, 12 * NB),
                       14: slice(12 * NB, 15 * NB),
                       15: slice(15 * NB, 16 * NB)}
            if 0 <= k - 1 < NTILES:
                j = k - 1
                bsl = slice(j * NB, (j + 1) * NB)
                emit_reduce(out0[:, bsl], r0_ref.pop(j), H1, "red0_",
                            fast=(j == NTILES - 1))
                if j in blocks0:
                    osl = blocks0[j]
                    nc.sync.dma_start(out=out_d.ap()[0:H1, osl],
                                      in_=out0[:, osl])
            if 0 <= k - 2 < NTILES:
                j = k - 2
                bsl = slice(j * NB, (j + 1) * NB)
                emit_reduce(out1[:, bsl], r1_ref.pop(j), L1, "red1_",
                            fast=(j == NTILES - 1))
                if j in blocks1:
                    osl = blocks1[j]
                    nc.sync.dma_start(out=out_d.ap()[H1:, osl],
                                      in_=out1[:, osl])

            # ---- S3(prev) 1:1 interleaved with S2(cur) + S4(prev) ----
            prod = [("v1", i) for i in range(NC1)] if prev is not None else []
            cons = []
            if cur is not None:
                cons += [("c0", i) for i in range(NC0)] + [("g0", 0)]
            if prev is not None:
                sq1_ref[prev] = sqp.tile([128, NC1 * NT], f16, tag="sq1",
                                         name="sq1")
                y1_ref[prev] = yps.tile([L1, NT], f32, tag="y1", name="y1")
                cons += [("c1", i) for i in range(NC1)] + [("g1", 0)]

            seq = []
            for j in range(max(len(prod), len(cons))):
                if j < len(prod):
                    seq.append(prod[j])
                if j < len(cons):
                    seq.append(cons[j])

            for op, i in seq:
                if op == "v1":
                    o, kc = CH1[i]
                    tprev = tb[prev % 3]
                    ps = sps.tile([128, NT], f32, tag="ps", name="ps")
                    nc.tensor.matmul(ps[0:kc, :], V1T[:, i * 128:i * 128 + kc],
                                     tprev[:], start=True, stop=True)
                    emit_square(sq1_ref[prev][0:kc, i * NT:(i + 1) * NT],
                                ps, kc, 6 + i)
                elif op == "c0":
                    o, kc = CH0[i]
                    nc.tensor.matmul(y0_ref[cur][:],
                                     C0T[0:kc, i * 128:(i + 1) * 128],
                                     sq0_ref[cur][0:kc, i * NT:(i + 1) * NT],
                                     start=(i == 0), stop=False)
                elif op == "g0":
                    t = tb[cur % 3]
                    t2 = t2b[cur % 3]
                    y0 = y0_ref[cur]
                    nc.tensor.matmul(y0[:], G0T[:], t2[:],
                                     start=False, stop=True)
                    # relu+bias: nh half into t rows 64:128, direct half to r0
                    nc.scalar.activation(t[NH0:T1, :], y0[0:H1, :], Relu,
                                         bias=b0t[0:H1])
                    r0 = rp.tile([H1, NT], f16, tag="r0", name="r0")
                    nc.scalar.activation(r0[:], y0[H1:L0, :], Relu,
                                         bias=b0t[H1:L0])
                    r0_ref[cur] = r0
                elif op == "c1":
                    o, kc = CH1[i]
                    nc.tensor.matmul(y1_ref[prev][:],
                                     C1T[0:kc, i * 128:(i + 1) * 128],
                                     sq1_ref[prev][0:kc, i * NT:(i + 1) * NT],
                                     start=(i == 0), stop=False)
                elif op == "g1":
                    y1 = y1_ref[prev]
                    nc.tensor.matmul(y1[:], G1T[:], t2b[prev % 3][:],
                                     start=False, stop=True)
                    r1 = rp.tile([L1, NT], f16, tag="r1", name="r1")
                    nc.scalar.activation(r1[:], y1[:], Relu, bias=b1t[:])
                    r1_ref[prev] = r1

            # the prologue iteration has no V1(prev) passes to cover the
            # relu-t latency before iteration 1's V1(0); keep PE busy (and
            # its p-state ramp alive) with a few dependency-free fillers
            if k == 0:
                warm2 = yps.tile([32, NT], f32, tag="y1", name="warm2")
                for _ in range(5):
                    nc.tensor.matmul(warm2[:], tb[2][32:NH0, 0:32],
                                     tb[2][32:NH0, :], start=True, stop=True)

            # prefetch x two tiles ahead (emitted after V1(prev)'s reads of
            # the same buffer slot so the WAR dependency lands correctly),
            # then the next tile's x^2 (its DMA landed last iteration)
            if cur is not None and cur + 2 < NTILES:
                nxt = cur + 2
                nc.sync.dma_start(out=tb[nxt % 3][0:F, :],
                                  in_=xT_d.ap()[:, nxt * NT:(nxt + 1) * NT])
            if cur is not None and cur + 1 < NTILES:
                nxt = cur + 1
                nc.gpsimd.tensor_mul(t2b[nxt % 3][:], tb[nxt % 3][0:F, :],
                                     tb[nxt % 3][0:F, :])

    nc.compile()
    _NC_CACHE[key] = nc
    return nc


def _run(inputs, trace=False):
    from concourse.bass_utils import run_bass_kernel_spmd

    x = np.asarray(inputs["x"], np.float32)
    w = _host_weights(inputs["W0"], inputs["b0"], inputs["W1"], inputs["b1"])
    nc = _build_nc()

    in_maps = []
    for c in range(NCORES):
        xs = x[c * BL:(c + 1) * BL]                          # [512, 39, 16]
        xT = np.ascontiguousarray(
            xs.transpose(1, 0, 2).reshape(F, NCOL)).astype(np.float16)
        m = {"xT": xT}
        m.update(w)
        in_maps.append(m)

    res = run_bass_kernel_spmd(nc, in_maps, core_ids=list(range(NCORES)),
                               trace=trace)
    out = np.empty((B, L0 - H1 + L1), np.float32)
    for c in range(NCORES):
        out[c * BL:(c + 1) * BL] = res.results[c]["out"].T.astype(np.float32)
    return out, res


def kernel(**inputs):
    out, _ = _run(inputs)
    return out


# revision 54
# speedup vs baseline: 1.2785x; 1.0076x over previous
"""Trainium2 Bass kernel for nn_CIN (xDeepFM compressed-interaction network).

Math: each CIN layer computes, per sample b and feature-dim d (a "column"
n=(b,d)):  y[o] = sum_{h,m} W[o,h,m] * a[h] * b[m]  — a bilinear form.

We avoid materializing the outer-product tensor z[h*m, n] (which needs slow
cross-partition broadcasts) by polarization:  a*b = ((a+b)^2 - a^2 - b^2)/2.
Each layer becomes:  s = V @ t   (pair sums, TensorE)
                     q = s*s     (elementwise square, ScalarE/VectorE)
                     y = C @ q + G @ t^2   (TensorE, PSUM-accumulated)
with V a 0/1 pair-selection matrix and C,G folded from W host-side (exact).

Layer 0 uses the symmetric fold (741 unordered pairs of 39 features);
layer 1 uses all 64*39=2496 (nh,x) pairs.  Everything on-device is fp16
(inputs/weights) with fp32 PSUM accumulation.

Schedule: software-pipelined across column tiles so TensorE never waits.
Per iteration k the PE stream is  V0(k) | V1(k-1) ⋈ [C0(k),G0(k),C1(k-1),
G1(k-1)]  (1:1 interleave of producer and consumer passes).  Squares are
split between ScalarE (odd chunks, activation-square straight from PSUM)
and VectorE (even chunks, tensor_mul(ps,ps)); relu/bias on ScalarE; the
d-axis reduction on VectorE.  PSUM budget: 6-deep ps ring + y0 + y1 =
8 banks exactly.

Sharding: pure data parallel — batch 4096 split as 512 per NeuronCore
across 8 cores; weights replicated.
"""

import numpy as np

B, F, D = 4096, 39, 16
L0, L1 = 128, 128
H1 = L0 // 2                      # 64 hidden maps feed layer 1
NCORES = 8
BL = B // NCORES                  # 512 samples per core
NCOL = BL * D                     # 8192 columns per core
NT = 512                          # columns per tile
NTILES = NCOL // NT               # 16
NB = NT // D                      # samples per tile (32)

K0 = F * (F - 1) // 2             # 741 layer-0 pairs
K1 = H1 * F                       # 2496 layer-1 pairs
T0 = F                            # t rows for layer 0 rhs (x)
T1 = 128                          # t rows: [x 0:39 | zeros 39:64 | nh 64:128]
NH0 = 64                          # nh base partition in t


def _chunks(k):
    out = []
    o = 0
    while o < k:
        c = min(128, k - o)
        out.append((o, c))
        o += k
        o = out[-1][0] + c
    return out


CH0 = _chunks(K0)                 # [(0,128)x5, (640,101)]
CH1 = _chunks(K1 + NH0)           # [(0,128)x20] — last 64 rows are nh^2
NC0 = len(CH0)
NC1 = len(CH1)


def _host_weights(W0, b0, W1, b1):
    """Fold W0/W1 into the square-trick operands (all exact, fp32)."""
    W0 = np.asarray(W0, np.float32)
    W1 = np.asarray(W1, np.float32)
    S0 = W0.reshape(L0, F, F)
    S0 = (S0 + S0.transpose(0, 2, 1)) / 2
    iu = np.triu_indices(F, 1)                       # 741 (h<m) pairs
    V0 = np.zeros((K0, F), np.float32)
    V0[np.arange(K0), iu[0]] = 1
    V0[np.arange(K0), iu[1]] = 1
    C0 = S0[:, iu[0], iu[1]]                         # [128, 741]
    rowsum = S0.sum(2)
    G0 = np.einsum('ohh->oh', S0) * 2 - rowsum       # S[h,h] - sum_{m!=h} S[h,m]

    B1 = W1.reshape(L1, H1, F)
    hh, mm = np.meshgrid(np.arange(H1), np.arange(F), indexing='ij')
    hh, mm = hh.ravel(), mm.ravel()                  # 2496 pairs, h-major
    # chunk 19 carries 64 extra pass-through rows (identity on nh) whose
    # squares are nh^2, so G1's nh^2 term rides in C1 and the separate
    # nh^2 elementwise op disappears.
    V1 = np.zeros((K1 + NH0, T1), np.float32)
    V1[np.arange(K1), mm] = 1                        # x part at rows 0:39
    V1[np.arange(K1), NH0 + hh] = 1                  # nh part at rows 64:128
    V1[K1 + np.arange(NH0), NH0 + np.arange(NH0)] = 1
    C1 = np.concatenate([B1[:, hh, mm] / 2,          # [128, 2496]
                         -B1.sum(2) / 2], axis=1)    # nh^2 coeffs [128, 64]
    G1 = (-B1.sum(1) / 2)                            # x^2 coeffs [128, 39]

    def pack_stationary(Ct, chunks):
        # Ct: [K, 128] -> packed [128, 128*nchunks] fp16, chunk i in
        # partitions 0:kc, free cols i*128:(i+1)*128
        out = np.zeros((128, 128 * len(chunks)), np.float16)
        for i, (o, kc) in enumerate(chunks):
            out[:kc, i * 128:i * 128 + 128] = Ct[o:o + kc, :]
        return out

    def pad_cols(Vt, n):
        out = np.zeros((Vt.shape[0], n), np.float16)
        out[:, :Vt.shape[1]] = Vt
        return out

    return {
        "V0T": pad_cols(V0.T, 128 * NC0),            # [39, 768]
        "V1T": pad_cols(V1.T, 128 * NC1),            # [128, 2560]
        "C0T": pack_stationary(C0.T, CH0),           # [128, 768]
        "C1T": pack_stationary(C1.T, CH1),           # [128, 2560]
        "G0T": G0.T.astype(np.float16),              # [39, 128]
        "G1T": G1.T.astype(np.float16),              # [39, 128]
        "b0": np.asarray(b0, np.float32).reshape(L0, 1),
        "b1": np.asarray(b1, np.float32).reshape(L1, 1),
    }


_NC_CACHE = {}


def _build_nc():
    key = "nc"
    if key in _NC_CACHE:
        return _NC_CACHE[key]
    from contextlib import ExitStack
    import concourse.bacc as bacc
    import concourse.mybir as mybir
    import concourse.tile as tile

    f16 = mybir.dt.float16
    f32 = mybir.dt.float32

    nc = bacc.Bacc("TRN2", target_bir_lowering=False, debug=False)

    xT_d = nc.dram_tensor("xT", [F, NCOL], f16, kind="ExternalInput")
    V0T_d = nc.dram_tensor("V0T", [F, 128 * NC0], f16, kind="ExternalInput")
    V1T_d = nc.dram_tensor("V1T", [T1, 128 * NC1], f16, kind="ExternalInput")
    C0T_d = nc.dram_tensor("C0T", [128, 128 * NC0], f16, kind="ExternalInput")
    C1T_d = nc.dram_tensor("C1T", [128, 128 * NC1], f16, kind="ExternalInput")
    G0T_d = nc.dram_tensor("G0T", [F, 128], f16, kind="ExternalInput")
    G1T_d = nc.dram_tensor("G1T", [F, 128], f16, kind="ExternalInput")
    b0_d = nc.dram_tensor("b0", [L0, 1], f32, kind="ExternalInput")
    b1_d = nc.dram_tensor("b1", [L1, 1], f32, kind="ExternalInput")
    out_d = nc.dram_tensor("out", [L0 - H1 + L1, BL], f16, kind="ExternalOutput")

    Relu = mybir.ActivationFunctionType.Relu

    # Engine assignment for the 26 per-tile squares (global index:
    # sq0 chunk i -> i, sq1 chunk i -> 6+i).  'A': ScalarE activation-square
    # straight from PSUM.  'D': VectorE copy to SBUF + in-place mul (PSUM
    # may only feed one DVE operand).  'P': VectorE copy + GPSIMD in-place
    # mul (only for chunks with long producer->consumer slack).
    SQ_ENG = {}
    for g in range(6):
        SQ_ENG[g] = 'D' if g % 2 == 0 else 'A'
    for i in range(20):
        SQ_ENG[6 + i] = 'D' if i in (0, 2, 5, 7, 10, 12, 15, 17) else 'A'

    with tile.TileContext(nc) as tc, ExitStack() as ctx:
        const = ctx.enter_context(tc.tile_pool(name="const", bufs=1))
        sqp = ctx.enter_context(tc.tile_pool(name="sqp", bufs=2))
        rp = ctx.enter_context(tc.tile_pool(name="rp", bufs=2))
        redp = ctx.enter_context(tc.tile_pool(name="redp", bufs=2))
        sps = ctx.enter_context(tc.tile_pool(name="sps", bufs=6, space="PSUM"))
        yps = ctx.enter_context(tc.tile_pool(name="yps", bufs=1, space="PSUM"))

        def emit_square(dst, ps, kc, g, force=None):
            eng = force or SQ_ENG[g]
            if eng == 'A':
                nc.scalar.square(dst, ps[0:kc, :])
            else:
                nc.vector.tensor_copy(dst, ps[0:kc, :])
                mul = nc.vector.tensor_mul if eng == 'D' else \
                    nc.gpsimd.tensor_mul
                mul(dst, dst, dst)

        def emit_reduce(out_ap, r, rows, tag, fast=False):
            # d-axis sum of relu'd maps; log2 add-tree on idle GPSIMD, or
            # a single DVE reduce when the result is on the drain path
            if fast:
                with nc.allow_low_precision(reason="16-term d-sum fits fp16"):
                    nc.vector.tensor_reduce(
                        out_ap, r[:].rearrange("p (b d) -> p b d", d=D),
                        axis=mybir.AxisListType.X, op=mybir.AluOpType.add)
                return
            v = r[:].rearrange("p (b d) -> p b d", d=D)
            for w in (8, 4, 2):
                tmp = redp.tile([rows, NB * w], f16, tag=f"{tag}{w}",
                                name=f"{tag}{w}")
                tv = tmp[:].rearrange("p (b d) -> p b d", d=w)
                nc.gpsimd.tensor_add(tv, v[:, :, 0:w], v[:, :, w:2 * w])
                v = tv
            nc.gpsimd.tensor_add(out_ap.rearrange("p (b d) -> p b d", d=1),
                                 v[:, :, 0:1], v[:, :, 1:2])

        # Weight loads, ordered so the first V0/C0 passes aren't gated on
        # the big layer-1 operands.
        V0T = const.tile([F, 128 * NC0], f16)
        nc.sync.dma_start(out=V0T[:], in_=V0T_d.ap())

        # x / x^2 working buffers (3-deep manual rotation; zero pad rows
        # 39:64 are written exactly once, here, off the critical engines).
        tb = [const.tile([T1, NT], f16, name=f"tb{i}") for i in range(3)]
        t2b = [const.tile([F, NT], f16, name=f"t2b{i}") for i in range(3)]
        for i in (2, 0, 1):
            # partition offsets must be 32-aligned; rows 32:39 are
            # overwritten by every x DMA, rows 39:64 stay zero forever.
            # tb[2] first: the warmup matmuls below read it, and its x DMA
            # comes latest.
            nc.gpsimd.memset(tb[i][32:NH0, :], 0.0)
        nc.sync.dma_start(out=tb[0][0:F, :], in_=xT_d.ap()[:, 0:NT])
        nc.sync.dma_start(out=tb[1][0:F, :], in_=xT_d.ap()[:, NT:2 * NT])

        C0T = const.tile([128, 128 * NC0], f16)
        nc.sync.dma_start(out=C0T[:], in_=C0T_d.ap())
        G0T = const.tile([F, 128], f16)
        nc.sync.dma_start(out=G0T[:], in_=G0T_d.ap())
        b0t = const.tile([L0, 1], f32)
        nc.sync.dma_start(out=b0t[:], in_=b0_d.ap())

        # Layer-1 weights are ~1.3 MB; split the transfers and order them
        # by first use so early passes aren't gated on the whole block.
        HC = 128 * NC1 // 2
        V1T = const.tile([T1, 128 * NC1], f16)
        C1T = const.tile([128, 128 * NC1], f16)
        nc.sync.dma_start(out=V1T[:, 0:HC], in_=V1T_d.ap()[:, 0:HC])
        nc.sync.dma_start(out=C1T[:, 0:HC], in_=C1T_d.ap()[:, 0:HC])
        nc.sync.dma_start(out=V1T[:, HC:], in_=V1T_d.ap()[:, HC:])
        nc.sync.dma_start(out=C1T[:, HC:], in_=C1T_d.ap()[:, HC:])
        G1T = const.tile([F, 128], f16)
        nc.sync.dma_start(out=G1T[:], in_=G1T_d.ap())
        b1t = const.tile([L1, 1], f32)
        nc.sync.dma_start(out=b1t[:], in_=b1_d.ap())

        nc.vector.tensor_mul(t2b[0][:], tb[0][0:F, :], tb[0][0:F, :])

        out0 = const.tile([H1, BL], f16)
        out1 = const.tile([L1, BL], f16)

        # Warmup matmuls on the zeroed pad rows: keeps PE continuously busy
        # through the input-DMA latency so the p-state ramp (0.65 GHz ->
        # 2.4 GHz after 3us of uninterrupted work) is already paid before
        # real work arrives.  Results land in the y1 PSUM slot, which is
        # overwritten (start=True) before its first real use.
        warm = yps.tile([32, NT], f32, tag="y1", name="warm")
        for _ in range(6):
            nc.tensor.matmul(warm[:], tb[2][32:NH0, 0:32],
                             tb[2][32:NH0, :], start=True, stop=True)

        r0_ref = {}
        r1_ref = {}
        sq0_ref = {}
        sq1_ref = {}
        y0_ref = {}
        y1_ref = {}

        for k in range(NTILES + 2):
            cur = k if k < NTILES else None
            prev = k - 1 if 1 <= k <= NTILES else None

            # ---- S1: V0(cur) + inline squares ----
            if cur is not None:
                t = tb[cur % 3]
                sq0 = sqp.tile([128, NC0 * NT], f16, tag="sq0", name="sq0")
                sq0_ref[cur] = sq0
                for i, (o, kc) in enumerate(CH0):
                    ps = sps.tile([128, NT], f32, tag="ps", name="ps")
                    nc.tensor.matmul(ps[0:kc, :], V0T[:, i * 128:i * 128 + kc],
                                     t[0:F, :], start=True, stop=True)
                    emit_square(sq0[0:kc, i * NT:(i + 1) * NT], ps, kc, i)
                y0_ref[cur] = yps.tile([L0, NT], f32, tag="y0", name="y0")

            # ---- d-axis reductions of finished tiles ----
            # stream finished output columns out in blocks, keeping the
            # final (drain-path) block tiny
            blocks0 = {3: slice(0, 4 * NB), 7: slice(4 * NB, 8 * NB),
                       11: slice(8 * NB, 12 * NB),
                       15: slice(12 * NB, 16 * NB)}
            blocks1 = {3: slice(0, 4 * NB), 7: slice(4 * NB, 8 * NB),
                       11: slice(8 * NB, 12 * NB),
                       14: slice(12 * NB, 15 * NB)}
            if 0 <= k - 1 < NTILES:
                j = k - 1
                bsl = slice(j * NB, (j + 1) * NB)
                emit_reduce(out0[:, bsl], r0_ref.pop(j), H1, "red0_",
                            fast=(j == NTILES - 1))
                if j in blocks0:
                    osl = blocks0[j]
                    nc.sync.dma_start(out=out_d.ap()[0:H1, osl],
                                      in_=out0[:, osl])
            if 0 <= k - 2 < NTILES - 1:
                j = k - 2
                bsl = slice(j * NB, (j + 1) * NB)
                emit_reduce(out1[:, bsl], r1_ref.pop(j), L1, "red1_")
                if j in blocks1:
                    osl = blocks1[j]
                    nc.sync.dma_start(out=out_d.ap()[H1:, osl],
                                      in_=out1[:, osl])

            # ---- S3(prev) 1:1 interleaved with S2(cur) + S4(prev) ----
            prod = [("v1", i) for i in range(NC1)] if prev is not None else []
            cons = []
            if cur is not None:
                cons += [("c0", i) for i in range(NC0)] + [("g0", 0)]
            if prev is not None:
                sq1_ref[prev] = sqp.tile([128, NC1 * NT], f16, tag="sq1",
                                         name="sq1")
                y1_ref[prev] = yps.tile([L1, NT], f32, tag="y1", name="y1")
                cons += [("c1", i) for i in range(NC1)] + [("g1", 0)]

            seq = []
            for j in range(max(len(prod), len(cons))):
                if j < len(prod):
                    seq.append(prod[j])
                if j < len(cons):
                    seq.append(cons[j])

            for op, i in seq:
                if op == "v1":
                    o, kc = CH1[i]
                    tprev = tb[prev % 3]
                    ps = sps.tile([128, NT], f32, tag="ps", name="ps")
                    nc.tensor.matmul(ps[0:kc, :], V1T[:, i * 128:i * 128 + kc],
                                     tprev[:], start=True, stop=True)
                    emit_square(sq1_ref[prev][0:kc, i * NT:(i + 1) * NT],
                                ps, kc, 6 + i)
                elif op == "c0":
                    o, kc = CH0[i]
                    nc.tensor.matmul(y0_ref[cur][:],
                                     C0T[0:kc, i * 128:(i + 1) * 128],
                                     sq0_ref[cur][0:kc, i * NT:(i + 1) * NT],
                                     start=(i == 0), stop=False)
                elif op == "g0":
                    t = tb[cur % 3]
                    t2 = t2b[cur % 3]
                    y0 = y0_ref[cur]
                    nc.tensor.matmul(y0[:], G0T[:], t2[:],
                                     start=False, stop=True)
                    # relu+bias: nh half into t rows 64:128, direct half to r0
                    nc.scalar.activation(t[NH0:T1, :], y0[0:H1, :], Relu,
                                         bias=b0t[0:H1])
                    r0 = rp.tile([H1, NT], f16, tag="r0", name="r0")
                    nc.scalar.activation(r0[:], y0[H1:L0, :], Relu,
                                         bias=b0t[H1:L0])
                    r0_ref[cur] = r0
                elif op == "c1":
                    o, kc = CH1[i]
                    nc.tensor.matmul(y1_ref[prev][:],
                                     C1T[0:kc, i * 128:(i + 1) * 128],
                                     sq1_ref[prev][0:kc, i * NT:(i + 1) * NT],
                                     start=(i == 0), stop=False)
                elif op == "g1":
                    y1 = y1_ref[prev]
                    nc.tensor.matmul(y1[:], G1T[:], t2b[prev % 3][:],
                                     start=False, stop=True)
                    r1 = rp.tile([L1, NT], f16, tag="r1", name="r1")
                    if prev == NTILES - 1:
                        # drain path: relu/reduce/DMA immediately
                        nc.scalar.activation(r1[:], y1[:], Relu, bias=b1t[:])
                        csl = slice(prev * NB, (prev + 1) * NB)
                        emit_reduce(out1[:, csl], r1, L1, "", fast=True)
                        nc.sync.dma_start(out=out_d.ap()[H1:, csl],
                                          in_=out1[:, csl])
                    else:
                        nc.scalar.activation(r1[:], y1[:], Relu, bias=b1t[:])
                        r1_ref[prev] = r1

            # the prologue iteration has no V1(prev) passes to cover the
            # relu-t latency before iteration 1's V1(0); keep PE busy (and
            # its p-state ramp alive) with a few dependency-free fillers
            if k == 0:
                warm2 = yps.tile([32, NT], f32, tag="y1", name="warm2")
                for _ in range(7):
                    nc.tensor.matmul(warm2[:], tb[2][32:NH0, 0:32],
                                     tb[2][32:NH0, :], start=True, stop=True)

            # prefetch x two tiles ahead (emitted after V1(prev)'s reads of
            # the same buffer slot so the WAR dependency lands correctly),
            # then the next tile's x^2 (its DMA landed last iteration)
            if cur is not None and cur + 2 < NTILES:
                nxt = cur + 2
                nc.sync.dma_start(out=tb[nxt % 3][0:F, :],
                                  in_=xT_d.ap()[:, nxt * NT:(nxt + 1) * NT])
            if cur is not None and cur + 1 < NTILES:
                nxt = cur + 1
                nc.gpsimd.tensor_mul(t2b[nxt % 3][:], tb[nxt % 3][0:F, :],
                                     tb[nxt % 3][0:F, :])

    nc.compile()
    _NC_CACHE[key] = nc
    return nc


def _run(inputs, trace=False):
    from concourse.bass_utils import run_bass_kernel_spmd

    x = np.asarray(inputs["x"], np.float32)
    w = _host_weights(inputs["W0"], inputs["b0"], inputs["W1"], inputs["b1"])
    nc = _build_nc()

    in_maps = []
    for c in range(NCORES):
        xs = x[c * BL:(c + 1) * BL]                          # [512, 39, 16]
        xT = np.ascontiguousarray(
            xs.transpose(1, 0, 2).reshape(F, NCOL)).astype(np.float16)
        m = {"xT": xT}
        m.update(w)
        in_maps.append(m)

    res = run_bass_kernel_spmd(nc, in_maps, core_ids=list(range(NCORES)),
                               trace=trace)
    out = np.empty((B, L0 - H1 + L1), np.float32)
    for c in range(NCORES):
        out[c * BL:(c + 1) * BL] = res.results[c]["out"].T.astype(np.float32)
    return out, res


def kernel(**inputs):
    out, _ = _run(inputs)
    return out


# revision 70
# speedup vs baseline: 1.2837x; 1.0040x over previous
"""Trainium2 Bass kernel for nn_CIN (xDeepFM compressed-interaction network).

Math: each CIN layer computes, per sample b and feature-dim d (a "column"
n=(b,d)):  y[o] = sum_{h,m} W[o,h,m] * a[h] * b[m]  — a bilinear form.

We avoid materializing the outer-product tensor z[h*m, n] (which needs slow
cross-partition broadcasts) by polarization:  a*b = ((a+b)^2 - a^2 - b^2)/2.
Each layer becomes:  s = V @ t   (pair sums, TensorE)
                     q = s*s     (elementwise square, ScalarE/VectorE)
                     y = C @ q + G @ t^2   (TensorE, PSUM-accumulated)
with V a 0/1 pair-selection matrix and C,G folded from W host-side (exact).

Layer 0 uses the symmetric fold (741 unordered pairs of 39 features);
layer 1 uses all 64*39=2496 (nh,x) pairs.  Everything on-device is fp16
(inputs/weights) with fp32 PSUM accumulation.

Schedule: software-pipelined across column tiles so TensorE never waits.
Per iteration k the PE stream is  V0(k) | V1(k-1) ⋈ [C0(k),G0(k),C1(k-1),
G1(k-1)]  (1:1 interleave of producer and consumer passes; 54 passes of
512 columns per tile, which is the PE floor for this algorithm: V passes
are output-row-bound, C passes contraction-bound, both at 128/pass).
Squares are split 15:11 between ScalarE (activation-square straight from
PSUM, 612 ns) and VectorE (tensor_copy + in-place mul — hardware allows
only one PSUM operand per DVE op); relu/bias on ScalarE; the d-axis
reductions run as log2 add-trees on the otherwise idle GPSIMD, x^2 too.
PSUM budget: 6-deep ps ring + y0 + y1 = 8 banks exactly.  Warmup
matmuls on zeroed pad rows hide the input-DMA latency and pre-pay the
PE p-state ramp; the V1 chunk-19 stationary carries a 64-row identity
block whose squares are nh^2 (folding G1's nh^2 term into C1); the
final tile's consumer passes run in column halves so the drain chain
(relu/reduce/DMA) is short and overlapped.

Sharding: pure data parallel — batch 4096 split as 512 per NeuronCore
across 8 cores; weights replicated.  Output returned fp16->fp32.
"""

import numpy as np

B, F, D = 4096, 39, 16
L0, L1 = 128, 128
H1 = L0 // 2                      # 64 hidden maps feed layer 1
NCORES = 8
BL = B // NCORES                  # 512 samples per core
NCOL = BL * D                     # 8192 columns per core
NT = 512                          # columns per tile
NTILES = NCOL // NT               # 16
NB = NT // D                      # samples per tile (32)
SPLIT = 384                       # uneven final-tile split: small drain half

K0 = F * (F - 1) // 2             # 741 layer-0 pairs
K1 = H1 * F                       # 2496 layer-1 pairs
T0 = F                            # t rows for layer 0 rhs (x)
T1 = 128                          # t rows: [x 0:39 | zeros 39:64 | nh 64:128]
NH0 = 64                          # nh base partition in t


def _chunks(k):
    out = []
    o = 0
    while o < k:
        c = min(128, k - o)
        out.append((o, c))
        o += k
        o = out[-1][0] + c
    return out


CH0 = _chunks(K0)                 # [(0,128)x5, (640,101)]
CH1 = _chunks(K1 + NH0)           # [(0,128)x20] — last 64 rows are nh^2
NC0 = len(CH0)
NC1 = len(CH1)


def _host_weights(W0, b0, W1, b1):
    """Fold W0/W1 into the square-trick operands (all exact, fp32)."""
    W0 = np.asarray(W0, np.float32)
    W1 = np.asarray(W1, np.float32)
    S0 = W0.reshape(L0, F, F)
    S0 = (S0 + S0.transpose(0, 2, 1)) / 2
    iu = np.triu_indices(F, 1)                       # 741 (h<m) pairs
    V0 = np.zeros((K0, F), np.float32)
    V0[np.arange(K0), iu[0]] = 1
    V0[np.arange(K0), iu[1]] = 1
    C0 = S0[:, iu[0], iu[1]]                         # [128, 741]
    rowsum = S0.sum(2)
    G0 = np.einsum('ohh->oh', S0) * 2 - rowsum       # S[h,h] - sum_{m!=h} S[h,m]

    B1 = W1.reshape(L1, H1, F)
    hh, mm = np.meshgrid(np.arange(H1), np.arange(F), indexing='ij')
    hh, mm = hh.ravel(), mm.ravel()                  # 2496 pairs, h-major
    # chunk 19 carries 64 extra pass-through rows (identity on nh) whose
    # squares are nh^2, so G1's nh^2 term rides in C1 and the separate
    # nh^2 elementwise op disappears.
    V1 = np.zeros((K1 + NH0, T1), np.float32)
    V1[np.arange(K1), mm] = 1                        # x part at rows 0:39
    V1[np.arange(K1), NH0 + hh] = 1                  # nh part at rows 64:128
    V1[K1 + np.arange(NH0), NH0 + np.arange(NH0)] = 1
    C1 = np.concatenate([B1[:, hh, mm] / 2,          # [128, 2496]
                         -B1.sum(2) / 2], axis=1)    # nh^2 coeffs [128, 64]
    G1 = (-B1.sum(1) / 2)                            # x^2 coeffs [128, 39]

    def pack_stationary(Ct, chunks):
        # Ct: [K, 128] -> packed [128, 128*nchunks] fp16, chunk i in
        # partitions 0:kc, free cols i*128:(i+1)*128
        out = np.zeros((128, 128 * len(chunks)), np.float16)
        for i, (o, kc) in enumerate(chunks):
            out[:kc, i * 128:i * 128 + 128] = Ct[o:o + kc, :]
        return out

    def pad_cols(Vt, n):
        out = np.zeros((Vt.shape[0], n), np.float16)
        out[:, :Vt.shape[1]] = Vt
        return out

    return {
        "V0T": pad_cols(V0.T, 128 * NC0),            # [39, 768]
        "V1T": pad_cols(V1.T, 128 * NC1),            # [128, 2560]
        "C0T": pack_stationary(C0.T, CH0),           # [128, 768]
        "C1T": pack_stationary(C1.T, CH1),           # [128, 2560]
        "G0T": G0.T.astype(np.float16),              # [39, 128]
        "G1T": G1.T.astype(np.float16),              # [39, 128]
        "b0": np.asarray(b0, np.float32).reshape(L0, 1),
        "b1": np.asarray(b1, np.float32).reshape(L1, 1),
    }


_NC_CACHE = {}


def _build_nc():
    key = "nc"
    if key in _NC_CACHE:
        return _NC_CACHE[key]
    from contextlib import ExitStack
    import concourse.bacc as bacc
    import concourse.mybir as mybir
    import concourse.tile as tile

    f16 = mybir.dt.float16
    f32 = mybir.dt.float32

    nc = bacc.Bacc("TRN2", target_bir_lowering=False, debug=False)

    xT_d = nc.dram_tensor("xT", [F, NCOL], f16, kind="ExternalInput")
    V0T_d = nc.dram_tensor("V0T", [F, 128 * NC0], f16, kind="ExternalInput")
    V1T_d = nc.dram_tensor("V1T", [T1, 128 * NC1], f16, kind="ExternalInput")
    C0T_d = nc.dram_tensor("C0T", [128, 128 * NC0], f16, kind="ExternalInput")
    C1T_d = nc.dram_tensor("C1T", [128, 128 * NC1], f16, kind="ExternalInput")
    G0T_d = nc.dram_tensor("G0T", [F, 128], f16, kind="ExternalInput")
    G1T_d = nc.dram_tensor("G1T", [F, 128], f16, kind="ExternalInput")
    b0_d = nc.dram_tensor("b0", [L0, 1], f32, kind="ExternalInput")
    b1_d = nc.dram_tensor("b1", [L1, 1], f32, kind="ExternalInput")
    out_d = nc.dram_tensor("out", [L0 - H1 + L1, BL], f16, kind="ExternalOutput")

    Relu = mybir.ActivationFunctionType.Relu

    # Engine assignment for the 26 per-tile squares (global index:
    # sq0 chunk i -> i, sq1 chunk i -> 6+i).  'A': ScalarE activation-square
    # straight from PSUM.  'D': VectorE copy to SBUF + in-place mul (PSUM
    # may only feed one DVE operand).  'P': VectorE copy + GPSIMD in-place
    # mul (only for chunks with long producer->consumer slack).
    SQ_ENG = {}
    for g in range(6):
        SQ_ENG[g] = 'D' if g % 2 == 0 else 'A'
    for i in range(20):
        SQ_ENG[6 + i] = 'D' if i in (0, 2, 5, 7, 10, 12, 15, 17) else 'A'

    with tile.TileContext(nc) as tc, ExitStack() as ctx:
        const = ctx.enter_context(tc.tile_pool(name="const", bufs=1))
        sqp = ctx.enter_context(tc.tile_pool(name="sqp", bufs=2))
        rp = ctx.enter_context(tc.tile_pool(name="rp", bufs=2))
        redp = ctx.enter_context(tc.tile_pool(name="redp", bufs=2))
        sps = ctx.enter_context(tc.tile_pool(name="sps", bufs=6, space="PSUM"))
        yps = ctx.enter_context(tc.tile_pool(name="yps", bufs=1, space="PSUM"))

        def emit_square(dst, ps, kc, g, force=None):
            eng = force or SQ_ENG[g]
            if eng == 'A':
                nc.scalar.square(dst, ps[0:kc, :])
            else:
                nc.vector.tensor_copy(dst, ps[0:kc, :])
                mul = nc.vector.tensor_mul if eng == 'D' else \
                    nc.gpsimd.tensor_mul
                mul(dst, dst, dst)

        def emit_reduce(out_ap, r, rows, tag, fast=False):
            # d-axis sum of relu'd maps; log2 add-tree on idle GPSIMD, or
            # a single DVE reduce when the result is on the drain path
            if fast:
                with nc.allow_low_precision(reason="16-term d-sum fits fp16"):
                    nc.vector.tensor_reduce(
                        out_ap, r[:].rearrange("p (b d) -> p b d", d=D),
                        axis=mybir.AxisListType.X, op=mybir.AluOpType.add)
                return
            v = r[:].rearrange("p (b d) -> p b d", d=D)
            for w in (8, 4, 2):
                tmp = redp.tile([rows, NB * w], f16, tag=f"{tag}{w}",
                                name=f"{tag}{w}")
                tv = tmp[:].rearrange("p (b d) -> p b d", d=w)
                nc.gpsimd.tensor_add(tv, v[:, :, 0:w], v[:, :, w:2 * w])
                v = tv
            nc.gpsimd.tensor_add(out_ap.rearrange("p (b d) -> p b d", d=1),
                                 v[:, :, 0:1], v[:, :, 1:2])

        # Weight loads, ordered so the first V0/C0 passes aren't gated on
        # the big layer-1 operands.
        V0T = const.tile([F, 128 * NC0], f16)
        nc.sync.dma_start(out=V0T[:], in_=V0T_d.ap())

        # x / x^2 working buffers (3-deep manual rotation; zero pad rows
        # 39:64 are written exactly once, here, off the critical engines).
        tb = [const.tile([T1, NT], f16, name=f"tb{i}") for i in range(3)]
        t2b = [const.tile([F, NT], f16, name=f"t2b{i}") for i in range(3)]
        for i in (2, 0, 1):
            # partition offsets must be 32-aligned; rows 32:39 are
            # overwritten by every x DMA, rows 39:64 stay zero forever.
            # tb[2] first: the warmup matmuls below read it, and its x DMA
            # comes latest.
            nc.gpsimd.memset(tb[i][32:NH0, :], 0.0)
        nc.sync.dma_start(out=tb[0][0:F, :], in_=xT_d.ap()[:, 0:NT])
        nc.sync.dma_start(out=tb[1][0:F, :], in_=xT_d.ap()[:, NT:2 * NT])

        C0T = const.tile([128, 128 * NC0], f16)
        nc.sync.dma_start(out=C0T[:], in_=C0T_d.ap())
        G0T = const.tile([F, 128], f16)
        nc.sync.dma_start(out=G0T[:], in_=G0T_d.ap())
        b0t = const.tile([L0, 1], f32)
        nc.sync.dma_start(out=b0t[:], in_=b0_d.ap())

        # Layer-1 weights are ~1.3 MB; split the transfers and order them
        # by first use so early passes aren't gated on the whole block.
        HC = 128 * NC1 // 2
        V1T = const.tile([T1, 128 * NC1], f16)
        C1T = const.tile([128, 128 * NC1], f16)
        nc.sync.dma_start(out=V1T[:, 0:HC], in_=V1T_d.ap()[:, 0:HC])
        nc.sync.dma_start(out=C1T[:, 0:HC], in_=C1T_d.ap()[:, 0:HC])
        nc.sync.dma_start(out=V1T[:, HC:], in_=V1T_d.ap()[:, HC:])
        nc.sync.dma_start(out=C1T[:, HC:], in_=C1T_d.ap()[:, HC:])
        G1T = const.tile([F, 128], f16)
        nc.sync.dma_start(out=G1T[:], in_=G1T_d.ap())
        b1t = const.tile([L1, 1], f32)
        nc.sync.dma_start(out=b1t[:], in_=b1_d.ap())

        nc.vector.tensor_mul(t2b[0][:], tb[0][0:F, :], tb[0][0:F, :])

        out0 = const.tile([H1, BL], f16)
        out1 = const.tile([L1, BL], f16)

        # Warmup matmuls on the zeroed pad rows: keeps PE continuously busy
        # through the input-DMA latency so the p-state ramp (0.65 GHz ->
        # 2.4 GHz after 3us of uninterrupted work) is already paid before
        # real work arrives.  Results land in the y1 PSUM slot, which is
        # overwritten (start=True) before its first real use.
        warm = yps.tile([32, NT], f32, tag="y1", name="warm")
        for _ in range(6):
            nc.tensor.matmul(warm[:], tb[2][32:NH0, 0:32],
                             tb[2][32:NH0, :], start=True, stop=True)

        r0_ref = {}
        r1_ref = {}
        sq0_ref = {}
        sq1_ref = {}
        y0_ref = {}
        y1_ref = {}

        for k in range(NTILES + 2):
            cur = k if k < NTILES else None
            prev = k - 1 if 1 <= k <= NTILES else None

            # ---- S1: V0(cur) + inline squares ----
            if cur is not None:
                t = tb[cur % 3]
                sq0 = sqp.tile([128, NC0 * NT], f16, tag="sq0", name="sq0")
                sq0_ref[cur] = sq0
                for i, (o, kc) in enumerate(CH0):
                    ps = sps.tile([128, NT], f32, tag="ps", name="ps")
                    nc.tensor.matmul(ps[0:kc, :], V0T[:, i * 128:i * 128 + kc],
                                     t[0:F, :], start=True, stop=True)
                    emit_square(sq0[0:kc, i * NT:(i + 1) * NT], ps, kc, i)
                y0_ref[cur] = yps.tile([L0, NT], f32, tag="y0", name="y0")

            # ---- d-axis reductions of finished tiles ----
            # stream finished output columns out in blocks, keeping the
            # final (drain-path) block tiny
            blocks0 = {3: slice(0, 4 * NB), 7: slice(4 * NB, 8 * NB),
                       11: slice(8 * NB, 12 * NB),
                       15: slice(12 * NB, 16 * NB)}
            blocks1 = {3: slice(0, 4 * NB), 7: slice(4 * NB, 8 * NB),
                       11: slice(8 * NB, 12 * NB),
                       14: slice(12 * NB, 15 * NB)}
            if 0 <= k - 1 < NTILES:
                j = k - 1
                bsl = slice(j * NB, (j + 1) * NB)
                emit_reduce(out0[:, bsl], r0_ref.pop(j), H1, "red0_",
                            fast=(j == NTILES - 1))
                if j in blocks0:
                    osl = blocks0[j]
                    nc.sync.dma_start(out=out_d.ap()[0:H1, osl],
                                      in_=out0[:, osl])
            if 0 <= k - 2 < NTILES - 1:
                j = k - 2
                bsl = slice(j * NB, (j + 1) * NB)
                emit_reduce(out1[:, bsl], r1_ref.pop(j), L1, "red1_")
                if j in blocks1:
                    osl = blocks1[j]
                    nc.sync.dma_start(out=out_d.ap()[H1:, osl],
                                      in_=out1[:, osl])

            # ---- S3(prev) 1:1 interleaved with S2(cur) + S4(prev) ----
            prod = [("v1", i) for i in range(NC1)] if prev is not None else []
            cons = []
            if cur is not None:
                cons += [("c0", i) for i in range(NC0)] + [("g0", 0)]
            if prev is not None:
                sq1_ref[prev] = sqp.tile([128, NC1 * NT], f16, tag="sq1",
                                         name="sq1")
                if prev == NTILES - 1:
                    # final tile: consume in column halves so the drain
                    # (relu/reduce/DMA) runs on half-width and the first
                    # half's drain overlaps the second half's compute
                    y1_ref[prev] = [
                        yps.tile([L1, SPLIT], f32, tag="y1", name="y1a"),
                        yps.tile([L1, NT - SPLIT], f32, tag="y0",
                                 name="y1b")]
                    cons += [("c1h", (i, 0)) for i in range(NC1)]
                    cons += [("g1h", 0)]
                    cons += [("c1h", (i, 1)) for i in range(NC1)]
                    cons += [("g1h", 1)]
                else:
                    y1_ref[prev] = yps.tile([L1, NT], f32, tag="y1",
                                            name="y1")
                    cons += [("c1", i) for i in range(NC1)] + [("g1", 0)]

            # with no cur-tile C0 passes to absorb the square latency, give
            # the V1 producers a head start before consuming
            lead = 4 if (cur is None and prev is not None) else 0
            seq = list(prod[:lead])
            prod_rest = prod[lead:]
            for j in range(max(len(prod_rest), len(cons))):
                if j < len(prod_rest):
                    seq.append(prod_rest[j])
                if j < len(cons):
                    seq.append(cons[j])

            r1_last = None
            for op, i in seq:
                if op == "v1":
                    o, kc = CH1[i]
                    tprev = tb[prev % 3]
                    ps = sps.tile([128, NT], f32, tag="ps", name="ps")
                    nc.tensor.matmul(ps[0:kc, :], V1T[:, i * 128:i * 128 + kc],
                                     tprev[:], start=True, stop=True)
                    emit_square(sq1_ref[prev][0:kc, i * NT:(i + 1) * NT],
                                ps, kc, 6 + i)
                elif op == "c0":
                    o, kc = CH0[i]
                    nc.tensor.matmul(y0_ref[cur][:],
                                     C0T[0:kc, i * 128:(i + 1) * 128],
                                     sq0_ref[cur][0:kc, i * NT:(i + 1) * NT],
                                     start=(i == 0), stop=False)
                elif op == "g0":
                    t = tb[cur % 3]
                    t2 = t2b[cur % 3]
                    y0 = y0_ref[cur]
                    nc.tensor.matmul(y0[:], G0T[:], t2[:],
                                     start=False, stop=True)
                    # relu+bias: nh half into t rows 64:128, direct half to r0
                    nc.scalar.activation(t[NH0:T1, :], y0[0:H1, :], Relu,
                                         bias=b0t[0:H1])
                    r0 = rp.tile([H1, NT], f16, tag="r0", name="r0")
                    nc.scalar.activation(r0[:], y0[H1:L0, :], Relu,
                                         bias=b0t[H1:L0])
                    r0_ref[cur] = r0
                elif op == "c1":
                    o, kc = CH1[i]
                    nc.tensor.matmul(y1_ref[prev][:],
                                     C1T[0:kc, i * 128:(i + 1) * 128],
                                     sq1_ref[prev][0:kc, i * NT:(i + 1) * NT],
                                     start=(i == 0), stop=False)
                elif op == "g1":
                    y1 = y1_ref[prev]
                    nc.tensor.matmul(y1[:], G1T[:], t2b[prev % 3][:],
                                     start=False, stop=True)
                    r1 = rp.tile([L1, NT], f16, tag="r1", name="r1")
                    nc.scalar.activation(r1[:], y1[:], Relu, bias=b1t[:])
                    r1_ref[prev] = r1
                elif op == "c1h":
                    ci, h = i
                    o, kc = CH1[ci]
                    hs = slice(0, SPLIT) if h == 0 else slice(SPLIT, NT)
                    nc.tensor.matmul(
                        y1_ref[prev][h][:],
                        C1T[0:kc, ci * 128:(ci + 1) * 128],
                        sq1_ref[prev][0:kc, ci * NT + hs.start:
                                      ci * NT + hs.stop],
                        start=(ci == 0), stop=False)
                elif op == "g1h":
                    h = i
                    hs = slice(0, SPLIT) if h == 0 else slice(SPLIT, NT)
                    y1h = y1_ref[prev][h]
                    nc.tensor.matmul(y1h[:], G1T[:], t2b[prev % 3][:, hs],
                                     start=False, stop=True)
                    r1 = rp.tile([L1, NT], f16, tag="r1", name="r1h") \
                        if h == 0 else r1_last
                    r1_last = r1
                    nc.scalar.activation(r1[:, hs], y1h[:], Relu,
                                         bias=b1t[:])
                    csl = slice(prev * NB + (0 if h == 0 else SPLIT // D),
                                prev * NB + (SPLIT // D if h == 0 else NB))
                    emit_reduce(out1[:, csl], r1[:, hs], L1, "", fast=True)
                    if h == 1:
                        csl = slice(prev * NB, (prev + 1) * NB)
                        nc.sync.dma_start(out=out_d.ap()[H1:, csl],
                                          in_=out1[:, csl])

            # the prologue iteration has no V1(prev) passes to cover the
            # relu-t latency before iteration 1's V1(0); keep PE busy (and
            # its p-state ramp alive) with a few dependency-free fillers
            if k == 0:
                warm2 = yps.tile([32, NT], f32, tag="y1", name="warm2")
                for _ in range(7):
                    nc.tensor.matmul(warm2[:], tb[2][32:NH0, 0:32],
                                     tb[2][32:NH0, :], start=True, stop=True)

            # prefetch x two tiles ahead (emitted after V1(prev)'s reads of
            # the same buffer slot so the WAR dependency lands correctly),
            # then the next tile's x^2 (its DMA landed last iteration)
            if cur is not None and cur + 2 < NTILES:
                nxt = cur + 2
                nc.sync.dma_start(out=tb[nxt % 3][0:F, :],
                                  in_=xT_d.ap()[:, nxt * NT:(nxt + 1) * NT])
            if cur is not None and cur + 1 < NTILES:
                nxt = cur + 1
                nc.gpsimd.tensor_mul(t2b[nxt % 3][:], tb[nxt % 3][0:F, :],
                                     tb[nxt % 3][0:F, :])

    nc.compile()
    _NC_CACHE[key] = nc
    return nc


def _run(inputs, trace=False):
    from concourse.bass_utils import run_bass_kernel_spmd

    x = np.asarray(inputs["x"], np.float32)
    w = _host_weights(inputs["W0"], inputs["b0"], inputs["W1"], inputs["b1"])
    nc = _build_nc()

    in_maps = []
    for c in range(NCORES):
        xs = x[c * BL:(c + 1) * BL]                          # [512, 39, 16]
        xT = np.ascontiguousarray(
            xs.transpose(1, 0, 2).reshape(F, NCOL)).astype(np.float16)
        m = {"xT": xT}
        m.update(w)
        in_maps.append(m)

    res = run_bass_kernel_spmd(nc, in_maps, core_ids=list(range(NCORES)),
                               trace=trace)
    out = np.empty((B, L0 - H1 + L1), np.float32)
    for c in range(NCORES):
        out[c * BL:(c + 1) * BL] = res.results[c]["out"].T.astype(np.float32)
    return out, res


def kernel(**inputs):
    out, _ = _run(inputs)
    return out
